# revision 12
# baseline (speedup 1.0000x reference)
"""Trainium2 Bass kernel for the tag-routed LSTM (moe_routing problem).

Strategy (data-parallel, zero cross-core comm in the recurrence):
  - 8 cores, core c owns batch rows [32c, 32c+32).
  - All 16 LSTM weight banks in bf16. The most-frequent banks stay resident
    in SBUF; the rest are streamed from DRAM on a compile-time prefetch
    schedule (the tag sequence is baked into the program at build time).
  - Phase 1 (device): Zx[t] = W_ih_aug[tag_t] @ embT_t (bias folded in as an
    extra K row), computed tag-grouped with N up to 512, stored bf16 in DRAM
    in the packed PSUM layout.
  - Phase 2 (serial recurrence): per step, one identity-matmul injects Zx
    into PSUM, then 64 bf16 matmuls add the W_hh part; ScalarE/VectorE apply
    the LSTM cell. h carried in bf16, c in fp32.
  - Packed layout: PSUM [128, 512] where M-tile m = (gate G=m//4, hidden
    tile ht=m%4) sits at cols [32m, 32m+32); h/c live as [128, (ktile, b)].

kernel(**inputs) -> (out, h, c) matching reference.py.
"""
import sys
import types
import warnings

sys.path.insert(0, "/opt/trn_rl_repo")
warnings.filterwarnings("ignore")
import numpy as np

if "antenv.axon_hooks" not in sys.modules:
    _hm = types.ModuleType("antenv.axon_hooks")
    _hm._hook = None
    _hm.set_axon_ntff_profile_hook = lambda x: setattr(_hm, "_hook", x)
    _hm.get_axon_ntff_profile_hook = lambda: _hm._hook
    sys.modules["antenv.axon_hooks"] = _hm
    try:
        from trn_agent_boot.trn_boot import _ntff_profile_via_ctypes
        _hm._hook = _ntff_profile_via_ctypes("/opt/axon/libaxon_pjrt.so")
    except Exception:
        pass

import concourse.bass as bass
import concourse.mybir as mybir
from concourse import bass_utils
try:
    from concourse.compiler_utils import get_compiler_flags, set_compiler_flags
    set_compiler_flags([
        f.replace("--enable-ldw-opt=false", "--enable-ldw-opt=true")
        for f in get_compiler_flags()
    ])
except Exception:
    pass

F32 = mybir.dt.float32
BF16 = mybir.dt.bfloat16
NPBF16 = mybir.dt.np(BF16)
SIG = mybir.ActivationFunctionType.Sigmoid
TANH = mybir.ActivationFunctionType.Tanh

NCORES = 8
B, H, E, T = 256, 512, 300, 16
BL = B // NCORES       # 32 batch rows per core
EK = 304               # padded x-part K: 300 emb + 1 bias + 3 zero
KT = [128, 128, 48]    # x-part K-tile sizes
NMT = 16               # M tiles (2048 gate rows / 128)
NKT = 4                # W_hh K tiles
BANK_W = NKT * NMT * 128   # 8192 cols per W_hh bank (lhsT layout)
WIH_W = 3 * NMT * 128      # 6144 cols per W_ih bank (lhsT layout)
R_RES = 7              # resident W_hh banks
N_STREAM = 3           # stream buffers for missing banks
CHUNK = 512


def host_prep(core, x, tags, emb, W_ih, W_hh, b_ih, b_hh, h0, c0, pos_of_step):
    S = tags.shape[0]
    bs = slice(BL * core, BL * (core + 1))
    # embT tag-grouped: [S, EK, BL] bf16; position pos_of_step[t] holds step t
    embT = np.zeros((S, EK, BL), dtype=NPBF16)
    gath = emb[np.asarray(x)[bs, :S]]                       # [BL, S, E]
    gathT = np.ascontiguousarray(gath.transpose(1, 2, 0))   # [S, E, BL]
    perm = np.empty(S, dtype=np.int64)
    for t in range(S):
        perm[pos_of_step[t]] = t
    embT[:, :E, :] = gathT[perm].astype(NPBF16)
    embT[:, E, :] = np.ones((), dtype=NPBF16)
    # W_hh lhsT: [16, 128, BANK_W]; col=(kc*16+m)*128+mm,
    # val = W_hh[b, G*512+ht*128+mm, kc*128+kp], m = G*4+ht
    Whh = W_hh.reshape(T, 4, 4, 128, 4, 128)  # [b, G, ht, mm, kc, kp]
    Whh = Whh[:, [2, 0, 1, 3]]  # M-tile gate order (g, i, f, o)
    whh = np.ascontiguousarray(
        Whh.transpose(0, 5, 4, 1, 2, 3).reshape(T, 128, BANK_W)
    ).astype(NPBF16)
    # W_ih_aug lhsT: [16, 128, WIH_W] (kc tile 2: 48 valid K rows)
    wih = np.zeros((T, 128, WIH_W), dtype=NPBF16)
    Waug = np.zeros((T, EK, 2048), dtype=np.float32)
    Waug[:, :E, :] = np.transpose(W_ih, (0, 2, 1))
    Waug[:, E, :] = b_ih + b_hh
    Waug = Waug.reshape(T, EK, 4, 512)[:, :, [2, 0, 1, 3]].reshape(T, EK, 2048)
    for kc in range(3):
        n = KT[kc]
        blk = Waug[:, 128 * kc:128 * kc + n, :]  # [b, n, 2048]
        wih[:, :n, 2048 * kc:2048 * (kc + 1)] = blk.astype(NPBF16)
    # h0/c0 packed [128, (ktile, b)]
    h0p = np.zeros((128, 128), dtype=NPBF16)
    c0p = np.zeros((128, 128), dtype=np.float32)
    h0c, c0c = h0[bs], c0[bs]
    for k in range(4):
        h0p[:, 32 * k:32 * k + BL] = h0c[:, 128 * k:128 * (k + 1)].T.astype(NPBF16)
        c0p[:, 32 * k:32 * k + BL] = c0c[:, 128 * k:128 * (k + 1)].T
    ident = np.eye(128, dtype=NPBF16)
    return {"embt": embT, "whh": whh, "wih": wih, "h0p": h0p, "c0p": c0p,
            "ident": ident}


def build_program(tags):
    tags = [int(t) for t in tags]
    S = len(tags)

    order = sorted(range(S), key=lambda t: (tags[t], t))
    pos_of_step = [0] * S
    for p, t in enumerate(order):
        pos_of_step[t] = p
    steps_of_tag = [[] for _ in range(T)]
    for t in range(S):
        steps_of_tag[tags[t]].append(t)

    freq = sorted(range(T), key=lambda b: -len(steps_of_tag[b]))
    resident = {b: i for i, b in enumerate(freq[:R_RES])}
    miss_steps = [t for t in range(S) if tags[t] not in resident]
    miss_idx_of_step = {t: j for j, t in enumerate(miss_steps)}

    # ---- phase-1 chunk table ----
    # chunk: (tau, lo, w, pos_base, wih_slot, new_tag, n1_cum)
    p1_chunks = []
    n1 = 0
    slot_of_tag = {}
    tag_order = []
    for tau in range(T):
        n = len(steps_of_tag[tau]) * BL
        if n == 0:
            continue
        pos_base = pos_of_step[steps_of_tag[tau][0]]
        new = True
        lo = 0
        while lo < n:
            w = min(CHUNK, n - lo)
            if new:
                slot_of_tag[tau] = len(tag_order) % 2
                tag_order.append(tau)
                n1 += 1  # wih dma
            n1 += 3      # emb dmas
            p1_chunks.append(
                dict(tau=tau, lo=lo, w=w, pos_base=pos_base,
                     wih_slot=slot_of_tag[tau], new_tag=new, n1_cum=n1)
            )
            new = False
            lo += w
    NCH = len(p1_chunks)
    # groups of the tag two-back boundary for wih reuse gating
    groups_before_tag = {}
    g = 0
    for ch in p1_chunks:
        if ch["new_tag"]:
            groups_before_tag[ch["tau"]] = g
        g += NMT

    # ---- phase-2 sync load sequence (FIFO) ----
    ld_seq = []
    for t in range(min(2, S)):
        ld_seq.append(("zx", t))
    for j, ms in enumerate(miss_steps):
        if ms <= 2:
            ld_seq.append(("miss", j))
    for u in range(S):
        if u + 2 < S:
            ld_seq.append(("zx", u + 2))
        for j, ms in enumerate(miss_steps):
            if ms >= 3 and ms - 3 == u:
                ld_seq.append(("miss", j))
    ld_index = {e: i for i, e in enumerate(ld_seq)}
    pe_ld_target = []
    for t in range(S):
        idx = ld_index[("zx", t)]
        if t in miss_idx_of_step:
            idx = max(idx, ld_index[("miss", miss_idx_of_step[t])])
        pe_ld_target.append(16 * (idx + 1))

    nc = bass.Bass("TRN2", target_bir_lowering=False, debug=False,
                   num_devices=NCORES)

    embt_d = nc.dram_tensor("embt", [S, EK, BL], BF16, kind="ExternalInput")
    whh_d = nc.dram_tensor("whh", [T, 128, BANK_W], BF16, kind="ExternalInput")
    wih_d = nc.dram_tensor("wih", [T, 128, WIH_W], BF16, kind="ExternalInput")
    h0p_d = nc.dram_tensor("h0p", [128, 128], BF16, kind="ExternalInput")
    c0p_d = nc.dram_tensor("c0p", [128, 128], F32, kind="ExternalInput")
    ident_d = nc.dram_tensor("ident", [128, 128], BF16, kind="ExternalInput")
    zx_d = nc.dram_tensor("zx", [S, 128, 512], BF16)
    hout_d = nc.dram_tensor("hout", [128, 128], BF16, kind="ExternalOutput")
    cout_d = nc.dram_tensor("cout", [128, 128], F32, kind="ExternalOutput")

    import contextlib
    ctx = contextlib.ExitStack()
    # persistent SBUF
    whh_res = ctx.enter_context(nc.sbuf_tensor("whh_res", [128, R_RES * BANK_W], BF16))
    ident_sb = ctx.enter_context(nc.sbuf_tensor("ident_sb", [128, 128], BF16))
    hT = ctx.enter_context(nc.sbuf_tensor("hT", [128, 128], BF16))
    c_st = ctx.enter_context(nc.sbuf_tensor("c_st", [128, 128], F32))
    sig_if = ctx.enter_context(nc.sbuf_tensor("sig_if", [128, 256], F32))
    sig_o = ctx.enter_context(nc.sbuf_tensor("sig_o", [128, 128], F32))
    tg = ctx.enter_context(nc.sbuf_tensor("tg", [128, 128], F32))
    tmp1 = ctx.enter_context(nc.sbuf_tensor("tmp1", [128, 128], F32))
    tmp2 = ctx.enter_context(nc.sbuf_tensor("tmp2", [128, 128], F32))
    tanhc = ctx.enter_context(nc.sbuf_tensor("tanhc", [128, 128], F32))

    # semaphores
    s_pre = ctx.enter_context(nc.semaphore("s_pre"))
    wih_sem = [ctx.enter_context(nc.semaphore(f"wih_sem{i}")) for i in range(2)]
    emb_sem = [ctx.enter_context(nc.semaphore(f"emb_sem{i}")) for i in range(2)]
    s1_mm = ctx.enter_context(nc.semaphore("s1_mm"))
    s1_cp = ctx.enter_context(nc.semaphore("s1_cp"))
    out_sem = [ctx.enter_context(nc.semaphore(f"out_sem{i}")) for i in range(2)]
    zx_sem = [ctx.enter_context(nc.semaphore(f"zx_sem{i}")) for i in range(2)]
    s_g = ctx.enter_context(nc.semaphore("s_g"))
    s_i2 = ctx.enter_context(nc.semaphore("s_i2"))
    s_if = ctx.enter_context(nc.semaphore("s_if"))
    miss_sem = [ctx.enter_context(nc.semaphore(f"miss_sem{i}"))
                for i in range(N_STREAM)]
    s_gates = ctx.enter_context(nc.semaphore("s_gates"))
    s_act = ctx.enter_context(nc.semaphore("s_act"))
    s_tc = ctx.enter_context(nc.semaphore("s_tc"))
    s_c = ctx.enter_context(nc.semaphore("s_c"))
    s_hnew = ctx.enter_context(nc.semaphore("s_hnew"))
    s_pfree = ctx.enter_context(nc.semaphore("s_pfree"))
    s_fin = ctx.enter_context(nc.semaphore("s_fin"))

    n_pre = 16 * (3 + R_RES)

    # =================== Block A: prologue + phase 1 ===================
    with contextlib.ExitStack() as p1ctx:
        ps1 = [p1ctx.enter_context(nc.psum_tensor(f"ps1{i}", [128, 512], F32))
               for i in range(2)]
        wih_sb = p1ctx.enter_context(nc.sbuf_tensor("wih_sb", [128, 2 * WIH_W], BF16))
        emb_sb = p1ctx.enter_context(nc.sbuf_tensor("emb_sb", [128, 2 * 3 * CHUNK], BF16))
        zxstage = p1ctx.enter_context(
            nc.sbuf_tensor("zxstage", [128, 2 * NMT * CHUNK], BF16))

        with nc.Block() as blockA:
            @blockA.sync
            def _(sync):
                sync.dma_start(ident_sb[:, :], ident_d[:, :]).then_inc(s_pre, 16)
                sync.dma_start(hT[:, :], h0p_d[:, :]).then_inc(s_pre, 16)
                sync.dma_start(c_st[:, :], c0p_d[:, :]).then_inc(s_pre, 16)
                for i, bk in enumerate(freq[:R_RES]):
                    sync.dma_start(
                        whh_res[:, i * BANK_W:(i + 1) * BANK_W], whh_d[bk, :, :]
                    ).then_inc(s_pre, 16)
                for ci, ch in enumerate(p1_chunks):
                    tau, lo, w = ch["tau"], ch["lo"], ch["w"]
                    ti = tag_order.index(tau)
                    if ch["new_tag"]:
                        if ti >= 2:
                            # wih slot reuse: PE done with all groups of the
                            # tag two back
                            sync.wait_ge(s1_mm, groups_before_tag[tag_order[ti - 1]])
                        sync.dma_start(
                            wih_sb[:, ch["wih_slot"] * WIH_W:(ch["wih_slot"] + 1) * WIH_W],
                            wih_d[tau, :, :],
                        ).then_inc(wih_sem[ti % 2], 16)
                    if ci >= 2:
                        # emb slot reuse: PE done with groups of chunk ci-2
                        sync.wait_ge(s1_mm, NMT * (ci - 1))
                    eslot = ci % 2
                    p_lo = ch["pos_base"] + lo // BL
                    npos = w // BL
                    for kc in range(3):
                        n = KT[kc]
                        sync.dma_start(
                            emb_sb.ap()
                            .rearrange("p (s c b) -> p s c b", s=2, c=3)[
                                :n, eslot, kc, 0:w]
                            .rearrange("p (s b) -> p s b", b=BL),
                            embt_d[p_lo:p_lo + npos, 128 * kc:128 * kc + n, :]
                            .rearrange("s p b -> p s b"),
                        ).then_inc(emb_sem[ci % 2], 16)
                    if ci >= 2:
                        co = p1_chunks[ci - 2]
                        sync.wait_ge(s1_cp, NMT * (ci - 1))
                        p_lo2 = co["pos_base"] + co["lo"] // BL
                        npos2 = co["w"] // BL
                        sync.dma_start(
                            zx_d[p_lo2:p_lo2 + npos2, :, :]
                            .rearrange("s p mb -> p s mb"),
                            zxstage.ap()
                            .rearrange("p (e s mb) -> p e s mb", e=2, s=NMT)[
                                :, (ci - 2) % 2, 0:npos2, :],
                        ).then_inc(out_sem[(ci - 2) % 2], 16)
                # drain tail
                for ci in range(max(0, NCH - 2), NCH):
                    co = p1_chunks[ci]
                    sync.wait_ge(s1_cp, NMT * (ci + 1))
                    p_lo2 = co["pos_base"] + co["lo"] // BL
                    npos2 = co["w"] // BL
                    sync.dma_start(
                        zx_d[p_lo2:p_lo2 + npos2, :, :]
                        .rearrange("s p mb -> p s mb"),
                        zxstage.ap()
                        .rearrange("p (e s mb) -> p e s mb", e=2, s=NMT)[
                            :, ci % 2, 0:npos2, :],
                    ).then_inc(out_sem[ci % 2], 16)

            @blockA.tensor
            def _(tensor):
                gidx = 0
                for ci, ch in enumerate(p1_chunks):
                    tau, lo, w = ch["tau"], ch["lo"], ch["w"]
                    ti = tag_order.index(tau)
                    tensor.wait_ge(wih_sem[ti % 2], 16 * (ti // 2 + 1))
                    tensor.wait_ge(emb_sem[ci % 2], 48 * (ci // 2 + 1))
                    slot, eslot = ch["wih_slot"], ci % 2
                    for m in range(NMT):
                        bank = ps1[gidx % 2]
                        if gidx >= 2:
                            tensor.wait_ge(s1_cp, gidx - 1)
                        for kc in range(3):
                            n = KT[kc]
                            mm = tensor.matmul(
                                bank[:, 0:w],
                                wih_sb[:n, slot * WIH_W + (kc * 16 + m) * 128:
                                       slot * WIH_W + (kc * 16 + m) * 128 + 128],
                                emb_sb[:n, eslot * 3 * CHUNK + kc * CHUNK:
                                       eslot * 3 * CHUNK + kc * CHUNK + w],
                                start=(kc == 0),
                                stop=(kc == 2),
                            )
                        mm.then_inc(s1_mm, 1)
                        gidx += 1

            @blockA.vector
            def _(vector):
                gidx = 0
                for ci, ch in enumerate(p1_chunks):
                    eslot = ci % 2
                    for m in range(NMT):
                        vector.wait_ge(s1_mm, gidx + 1)
                        if ci >= 2 and m == 0:
                            vector.wait_ge(out_sem[ci % 2], 16 * ((ci - 2) // 2 + 1))
                        npos = ch["w"] // BL
                        vector.tensor_copy(
                            zxstage.ap()
                            .rearrange("p (e s mb) -> p e s mb", e=2, s=NMT)[
                                :, eslot, 0:npos, m * BL:m * BL + BL],
                            ps1[gidx % 2]
                            .ap()
                            .rearrange("p (s b) -> p s b", b=BL)[:, 0:npos, :],
                        ).then_inc(s1_cp, 1)
                        gidx += 1

    # =================== Block B: phase 2 ===================
    with contextlib.ExitStack() as p2ctx:
        pg = [[p2ctx.enter_context(nc.psum_tensor(f"pg{e}_{q}", [128, 128], F32))
               for q in range(4)] for e in range(2)]
        whh_str = p2ctx.enter_context(
            nc.sbuf_tensor("whh_str", [128, N_STREAM * BANK_W], BF16))
        zx_sb = p2ctx.enter_context(nc.sbuf_tensor("zx_sb", [128, 2 * 512], BF16))

        def whh_src(t):
            tau = tags[t]
            if tau in resident:
                return whh_res, resident[tau] * BANK_W
            return whh_str, (miss_idx_of_step[t] % N_STREAM) * BANK_W

        with nc.Block() as blockB:
            @blockB.sync
            def _(sync):
                for sl in range(2):
                    if NCH - 2 + sl >= 0 and NCH > sl:
                        last_ci = max(c2 for c2 in range(NCH) if c2 % 2 == sl)
                        sync.wait_ge(out_sem[sl], 16 * (last_ci // 2 + 1))
                for kind, v in ld_seq:
                    if kind == "zx":
                        t = v
                        if t >= 2:
                            sync.wait_ge(s_gates, t - 1)
                        sync.dma_start(
                            zx_sb[:, (t % 2) * 512:(t % 2) * 512 + 512],
                            zx_d[pos_of_step[t], :, :],
                        ).then_inc(zx_sem[t % 2], 16)
                    else:
                        j = v
                        ms = miss_steps[j]
                        if j >= N_STREAM:
                            sync.wait_ge(s_gates, miss_steps[j - N_STREAM] + 1)
                        sync.dma_start(
                            whh_str[:, (j % N_STREAM) * BANK_W:
                                    (j % N_STREAM + 1) * BANK_W],
                            whh_d[tags[ms], :, :],
                        ).then_inc(miss_sem[j % N_STREAM], 16)
                sync.wait_ge(s_hnew, S)
                sync.wait_ge(s_c, S)
                sync.dma_start(hout_d[:, :], hT[:, :]).then_inc(s_fin, 16)
                sync.dma_start(cout_d[:, :], c_st[:, :]).then_inc(s_fin, 16)
                sync.wait_ge(s_fin, 32)

            @blockB.tensor
            def _(tensor):
                for t in range(S):
                    src, cbase = whh_src(t)
                    tensor.wait_ge(zx_sem[t % 2], 16 * (t // 2 + 1))
                    if t in miss_idx_of_step:
                        j = miss_idx_of_step[t]
                        tensor.wait_ge(miss_sem[j % N_STREAM],
                                       16 * (j // N_STREAM + 1))
                    if t == 0:
                        tensor.wait_ge(s_pre, n_pre)
                    if t >= 2:
                        tensor.wait_ge(s_pfree, t - 1)
                    banks = pg[t % 2]
                    for q in range(4):
                        tensor.matmul(
                            banks[q][:, :],
                            ident_sb[:, :],
                            zx_sb[:, (t % 2) * 512 + 128 * q:
                                  (t % 2) * 512 + 128 * q + 128],
                            start=True, stop=False, skip_group_check=True,
                        )
                    if t >= 1:
                        tensor.wait_ge(s_hnew, t)
                    for m in range(NMT):
                        for kc in range(NKT):
                            last = (m == NMT - 1 and kc == NKT - 1)
                            mm = tensor.matmul(
                                banks[m // 4][:, 32 * (m % 4):32 * (m % 4) + 32],
                                src[:, cbase + (kc * 16 + m) * 128:
                                    cbase + (kc * 16 + m) * 128 + 128],
                                hT[:, 32 * kc:32 * kc + 32],
                                start=False, stop=last, skip_group_check=True,
                            )
                            if kc == NKT - 1:
                                if m == 3:
                                    mm.then_inc(s_g, 1)
                                elif m == 7:
                                    mm.then_inc(s_i2, 1)
                                elif m == 11:
                                    mm.then_inc(s_if, 1)
                                elif m == NMT - 1:
                                    mm.then_inc(s_gates, 1)

            @blockB.scalar
            def _(scalar):
                for t in range(S):
                    banks = pg[t % 2]
                    scalar.wait_ge(s_g, t + 1)
                    scalar.activation(tg[:, :], banks[0][:, :], TANH).then_inc(s_act, 1)
                    scalar.wait_ge(s_i2, t + 1)
                    scalar.activation(sig_if[:, 0:128], banks[1][:, :], SIG).then_inc(s_act, 1)
                    scalar.wait_ge(s_if, t + 1)
                    scalar.activation(sig_if[:, 128:256], banks[2][:, :], SIG).then_inc(s_act, 1)
                    scalar.wait_ge(s_gates, t + 1)
                    scalar.activation(sig_o[:, :], banks[3][:, :], SIG).then_inc(s_pfree, 1)
                    scalar.wait_ge(s_c, t + 1)
                    scalar.activation(tanhc[:, :], c_st[:, :], TANH).then_inc(s_tc, 1)

            @blockB.vector
            def _(vector):
                for t in range(S):
                    vector.wait_ge(s_act, 3 * t + 2)
                    vector.tensor_mul(tmp2[:, :], sig_if[:, 0:128], tg[:, :])
                    vector.wait_ge(s_act, 3 * t + 3)
                    vector.tensor_mul(tmp1[:, :], sig_if[:, 128:256], c_st[:, :])
                    vector.tensor_add(c_st[:, :], tmp1[:, :], tmp2[:, :]).then_inc(s_c, 1)
                    vector.wait_ge(s_tc, t + 1)
                    vector.wait_ge(s_pfree, t + 1)
                    vector.tensor_mul(hT[:, :], sig_o[:, :], tanhc[:, :]).then_inc(s_hnew, 1)

    ctx.close()
    return nc, {"pos_of_step": pos_of_step, "S": S}


_CACHE = {}


def _get_program(tags_key):
    if tags_key not in _CACHE:
        _CACHE[tags_key] = build_program(list(tags_key))
    return _CACHE[tags_key]


def run_on_hw(x, tags, emb, W_ih, W_hh, b_ih, b_hh, h0, c0, trace=False):
    tags_key = tuple(int(t) for t in tags)
    nc, meta = _get_program(tags_key)
    pos = meta["pos_of_step"]
    in_maps = [
        host_prep(c, x, tags, emb, W_ih, W_hh, b_ih, b_hh, h0, c0, pos)
        for c in range(NCORES)
    ]
    res = bass_utils.run_bass_kernel_spmd(
        nc, in_maps, core_ids=list(range(NCORES)), trace=trace
    )
    S = meta["S"]
    h = np.zeros((B, H), dtype=np.float32)
    c = np.zeros((B, H), dtype=np.float32)
    for core in range(NCORES):
        hp = np.asarray(res.results[core]["hout"]).astype(np.float32)
        cp = np.asarray(res.results[core]["cout"])
        bs = slice(BL * core, BL * (core + 1))
        for k in range(4):
            h[bs, 128 * k:128 * (k + 1)] = hp[:, 32 * k:32 * k + BL].T
            c[bs, 128 * k:128 * (k + 1)] = cp[:, 32 * k:32 * k + BL].T
    return h, c, res


def kernel(x, tags, emb, W_ih, W_hh, b_ih, b_hh, fc_W, fc_b, h0, c0):
    x = np.asarray(x)
    tags = np.asarray(tags)
    emb = np.asarray(emb, dtype=np.float32)
    W_ih = np.asarray(W_ih, dtype=np.float32)
    W_hh = np.asarray(W_hh, dtype=np.float32)
    b_ih = np.asarray(b_ih, dtype=np.float32)
    b_hh = np.asarray(b_hh, dtype=np.float32)
    fc_W = np.asarray(fc_W, dtype=np.float32)
    fc_b = np.asarray(fc_b, dtype=np.float32)
    h0 = np.asarray(h0, dtype=np.float32)
    c0 = np.asarray(c0, dtype=np.float32)
    h, c, _ = run_on_hw(x, tags, emb, W_ih, W_hh, b_ih, b_hh, h0, c0)
    out = (1.0 / (1.0 + np.exp(-(h @ fc_W.T + fc_b)))).astype(np.float32)
    return out, h, c


# revision 13
# speedup vs baseline: 1.1587x; 1.1587x over previous
"""Trainium2 Bass kernel for the tag-routed LSTM (moe_routing problem).

Strategy (data-parallel, zero cross-core comm in the recurrence):
  - 8 cores, core c owns batch rows [32c, 32c+32).
  - All 16 LSTM weight banks in bf16. The most-frequent banks stay resident
    in SBUF; the rest are streamed from DRAM on a compile-time prefetch
    schedule (the tag sequence is baked into the program at build time).
  - Phase 1 (device): Zx[t] = W_ih_aug[tag_t] @ embT_t (bias folded in as an
    extra K row), computed tag-grouped with N up to 512, stored bf16 in DRAM
    in the packed PSUM layout.
  - Phase 2 (serial recurrence): per step, one identity-matmul injects Zx
    into PSUM, then 64 bf16 matmuls add the W_hh part; ScalarE/VectorE apply
    the LSTM cell. h carried in bf16, c in fp32.
  - Packed layout: PSUM [128, 512] where M-tile m = (gate G=m//4, hidden
    tile ht=m%4) sits at cols [32m, 32m+32); h/c live as [128, (ktile, b)].

kernel(**inputs) -> (out, h, c) matching reference.py.
"""
import sys
import types
import warnings

sys.path.insert(0, "/opt/trn_rl_repo")
warnings.filterwarnings("ignore")
import numpy as np

if "antenv.axon_hooks" not in sys.modules:
    _hm = types.ModuleType("antenv.axon_hooks")
    _hm._hook = None
    _hm.set_axon_ntff_profile_hook = lambda x: setattr(_hm, "_hook", x)
    _hm.get_axon_ntff_profile_hook = lambda: _hm._hook
    sys.modules["antenv.axon_hooks"] = _hm
    try:
        from trn_agent_boot.trn_boot import _ntff_profile_via_ctypes
        _hm._hook = _ntff_profile_via_ctypes("/opt/axon/libaxon_pjrt.so")
    except Exception:
        pass

import concourse.bass as bass
import concourse.mybir as mybir
from concourse import bass_utils
try:
    from concourse.compiler_utils import get_compiler_flags, set_compiler_flags
    set_compiler_flags([
        f.replace("--enable-ldw-opt=false", "--enable-ldw-opt=true")
        for f in get_compiler_flags()
    ])
except Exception:
    pass

F32 = mybir.dt.float32
BF16 = mybir.dt.bfloat16
NPBF16 = mybir.dt.np(BF16)
SIG = mybir.ActivationFunctionType.Sigmoid
TANH = mybir.ActivationFunctionType.Tanh

NCORES = 8
B, H, E, T = 256, 512, 300, 16
BL = B // NCORES       # 32 batch rows per core
EK = 304               # padded x-part K: 300 emb + 1 bias + 3 zero
KT = [128, 128, 48]    # x-part K-tile sizes
NMT = 16               # M tiles (2048 gate rows / 128)
NKT = 4                # W_hh K tiles
BANK_W = NKT * NMT * 128   # 8192 cols per W_hh bank (lhsT layout)
WIH_W = 3 * NMT * 128      # 6144 cols per W_ih bank (lhsT layout)
R_RES = 7              # resident W_hh banks
N_STREAM = 3           # stream buffers for missing banks
CHUNK = 512


def host_prep(core, x, tags, emb, W_ih, W_hh, b_ih, b_hh, h0, c0, pos_of_step):
    S = tags.shape[0]
    bs = slice(BL * core, BL * (core + 1))
    # embT tag-grouped: [S, EK, BL] bf16; position pos_of_step[t] holds step t
    embT = np.zeros((S, EK, BL), dtype=NPBF16)
    gath = emb[np.asarray(x)[bs, :S]]                       # [BL, S, E]
    gathT = np.ascontiguousarray(gath.transpose(1, 2, 0))   # [S, E, BL]
    perm = np.empty(S, dtype=np.int64)
    for t in range(S):
        perm[pos_of_step[t]] = t
    embT[:, :E, :] = gathT[perm].astype(NPBF16)
    embT[:, E, :] = np.ones((), dtype=NPBF16)
    # W_hh lhsT: [16, 128, BANK_W]; col=(kc*16+m)*128+mm,
    # val = W_hh[b, G*512+ht*128+mm, kc*128+kp], m = G*4+ht
    Whh = W_hh.reshape(T, 4, 4, 128, 4, 128)  # [b, G, ht, mm, kc, kp]
    Whh = Whh[:, [2, 0, 1, 3]]  # M-tile gate order (g, i, f, o)
    whh = np.ascontiguousarray(
        Whh.transpose(0, 5, 4, 1, 2, 3).reshape(T, 128, BANK_W)
    ).astype(NPBF16)
    # W_ih_aug lhsT: [16, 128, WIH_W] (kc tile 2: 48 valid K rows)
    wih = np.zeros((T, 128, WIH_W), dtype=NPBF16)
    Waug = np.zeros((T, EK, 2048), dtype=np.float32)
    Waug[:, :E, :] = np.transpose(W_ih, (0, 2, 1))
    Waug[:, E, :] = b_ih + b_hh
    Waug = Waug.reshape(T, EK, 4, 512)[:, :, [2, 0, 1, 3]].reshape(T, EK, 2048)
    for kc in range(3):
        n = KT[kc]
        blk = Waug[:, 128 * kc:128 * kc + n, :]  # [b, n, 2048]
        wih[:, :n, 2048 * kc:2048 * (kc + 1)] = blk.astype(NPBF16)
    # h0/c0 packed [128, (ktile, b)]
    h0p = np.zeros((128, 128), dtype=NPBF16)
    c0p = np.zeros((128, 128), dtype=np.float32)
    h0c, c0c = h0[bs], c0[bs]
    for k in range(4):
        h0p[:, 32 * k:32 * k + BL] = h0c[:, 128 * k:128 * (k + 1)].T.astype(NPBF16)
        c0p[:, 32 * k:32 * k + BL] = c0c[:, 128 * k:128 * (k + 1)].T
    ident = np.eye(128, dtype=NPBF16)
    return {"embt": embT, "whh": whh, "wih": wih, "h0p": h0p, "c0p": c0p,
            "ident": ident}


def build_program(tags):
    tags = [int(t) for t in tags]
    S = len(tags)

    order = sorted(range(S), key=lambda t: (tags[t], t))
    pos_of_step = [0] * S
    for p, t in enumerate(order):
        pos_of_step[t] = p
    steps_of_tag = [[] for _ in range(T)]
    for t in range(S):
        steps_of_tag[tags[t]].append(t)

    freq = sorted(range(T), key=lambda b: -len(steps_of_tag[b]))
    resident = {b: i for i, b in enumerate(freq[:R_RES])}
    miss_steps = [t for t in range(S) if tags[t] not in resident]
    miss_idx_of_step = {t: j for j, t in enumerate(miss_steps)}

    # ---- phase-1 chunk table ----
    # chunk: (tau, lo, w, pos_base, wih_slot, new_tag, n1_cum)
    p1_chunks = []
    n1 = 0
    slot_of_tag = {}
    tag_order = []
    for tau in range(T):
        n = len(steps_of_tag[tau]) * BL
        if n == 0:
            continue
        pos_base = pos_of_step[steps_of_tag[tau][0]]
        new = True
        lo = 0
        while lo < n:
            w = min(CHUNK, n - lo)
            if new:
                slot_of_tag[tau] = len(tag_order) % 2
                tag_order.append(tau)
                n1 += 1  # wih dma
            n1 += 3      # emb dmas
            p1_chunks.append(
                dict(tau=tau, lo=lo, w=w, pos_base=pos_base,
                     wih_slot=slot_of_tag[tau], new_tag=new, n1_cum=n1)
            )
            new = False
            lo += w
    NCH = len(p1_chunks)
    # groups of the tag two-back boundary for wih reuse gating
    groups_before_tag = {}
    g = 0
    for ch in p1_chunks:
        if ch["new_tag"]:
            groups_before_tag[ch["tau"]] = g
        g += NMT

    # ---- phase-2 sync load sequence (FIFO) ----
    ld_seq = []
    for t in range(min(2, S)):
        ld_seq.append(("zx", t))
    for j, ms in enumerate(miss_steps):
        if ms <= 2:
            ld_seq.append(("miss", j))
    for u in range(S):
        if u + 2 < S:
            ld_seq.append(("zx", u + 2))
        for j, ms in enumerate(miss_steps):
            if ms >= 3 and ms - 3 == u:
                ld_seq.append(("miss", j))
    ld_index = {e: i for i, e in enumerate(ld_seq)}
    pe_ld_target = []
    for t in range(S):
        idx = ld_index[("zx", t)]
        if t in miss_idx_of_step:
            idx = max(idx, ld_index[("miss", miss_idx_of_step[t])])
        pe_ld_target.append(16 * (idx + 1))

    nc = bass.Bass("TRN2", target_bir_lowering=False, debug=False,
                   num_devices=NCORES)

    embt_d = nc.dram_tensor("embt", [S, EK, BL], BF16, kind="ExternalInput")
    whh_d = nc.dram_tensor("whh", [T, 128, BANK_W], BF16, kind="ExternalInput")
    wih_d = nc.dram_tensor("wih", [T, 128, WIH_W], BF16, kind="ExternalInput")
    h0p_d = nc.dram_tensor("h0p", [128, 128], BF16, kind="ExternalInput")
    c0p_d = nc.dram_tensor("c0p", [128, 128], F32, kind="ExternalInput")
    ident_d = nc.dram_tensor("ident", [128, 128], BF16, kind="ExternalInput")
    zx_d = nc.dram_tensor("zx", [S, 128, 512], BF16)
    hout_d = nc.dram_tensor("hout", [128, 128], BF16, kind="ExternalOutput")
    cout_d = nc.dram_tensor("cout", [128, 128], F32, kind="ExternalOutput")

    import contextlib
    ctx = contextlib.ExitStack()
    # persistent SBUF
    whh_res = ctx.enter_context(nc.sbuf_tensor("whh_res", [128, R_RES * BANK_W], BF16))
    ident_sb = ctx.enter_context(nc.sbuf_tensor("ident_sb", [128, 128], BF16))
    hT = ctx.enter_context(nc.sbuf_tensor("hT", [128, 128], BF16))
    c_st = ctx.enter_context(nc.sbuf_tensor("c_st", [128, 128], F32))
    sig_if = ctx.enter_context(nc.sbuf_tensor("sig_if", [128, 256], F32))
    sig_o = ctx.enter_context(nc.sbuf_tensor("sig_o", [128, 128], F32))
    tg = ctx.enter_context(nc.sbuf_tensor("tg", [128, 128], F32))
    tmp1 = ctx.enter_context(nc.sbuf_tensor("tmp1", [128, 128], F32))
    tmp2 = ctx.enter_context(nc.sbuf_tensor("tmp2", [128, 128], F32))
    tanhc = ctx.enter_context(nc.sbuf_tensor("tanhc", [128, 128], F32))

    # semaphores
    s_pre = ctx.enter_context(nc.semaphore("s_pre"))
    wih_sem = [ctx.enter_context(nc.semaphore(f"wih_sem{i}")) for i in range(2)]
    emb_sem = [ctx.enter_context(nc.semaphore(f"emb_sem{i}")) for i in range(2)]
    s1_mm = ctx.enter_context(nc.semaphore("s1_mm"))
    s1_cp = ctx.enter_context(nc.semaphore("s1_cp"))
    out_sem = [ctx.enter_context(nc.semaphore(f"out_sem{i}")) for i in range(2)]
    zx_sem = [ctx.enter_context(nc.semaphore(f"zx_sem{i}")) for i in range(2)]
    s_zxd = ctx.enter_context(nc.semaphore("s_zxd"))
    s_g = ctx.enter_context(nc.semaphore("s_g"))
    s_i2 = ctx.enter_context(nc.semaphore("s_i2"))
    s_if = ctx.enter_context(nc.semaphore("s_if"))
    miss_sem = [ctx.enter_context(nc.semaphore(f"miss_sem{i}"))
                for i in range(N_STREAM)]
    s_gates = ctx.enter_context(nc.semaphore("s_gates"))
    s_act = ctx.enter_context(nc.semaphore("s_act"))
    s_tc = ctx.enter_context(nc.semaphore("s_tc"))
    s_c = ctx.enter_context(nc.semaphore("s_c"))
    s_hnew = ctx.enter_context(nc.semaphore("s_hnew"))
    s_pfree = ctx.enter_context(nc.semaphore("s_pfree"))
    s_fin = ctx.enter_context(nc.semaphore("s_fin"))

    n_pre = 16 * (3 + R_RES)

    # =================== Block A: prologue + phase 1 ===================
    with contextlib.ExitStack() as p1ctx:
        ps1 = [p1ctx.enter_context(nc.psum_tensor(f"ps1{i}", [128, 512], F32))
               for i in range(4)]
        wih_sb = p1ctx.enter_context(nc.sbuf_tensor("wih_sb", [128, 2 * WIH_W], BF16))
        emb_sb = p1ctx.enter_context(nc.sbuf_tensor("emb_sb", [128, 2 * 3 * CHUNK], BF16))
        zxstage = p1ctx.enter_context(
            nc.sbuf_tensor("zxstage", [128, 2 * NMT * CHUNK], BF16))

        with nc.Block() as blockA:
            @blockA.sync
            def _(sync):
                sync.dma_start(ident_sb[:, :], ident_d[:, :]).then_inc(s_pre, 16)
                sync.dma_start(hT[:, :], h0p_d[:, :]).then_inc(s_pre, 16)
                sync.dma_start(c_st[:, :], c0p_d[:, :]).then_inc(s_pre, 16)
                for i, bk in enumerate(freq[:R_RES]):
                    sync.dma_start(
                        whh_res[:, i * BANK_W:(i + 1) * BANK_W], whh_d[bk, :, :]
                    ).then_inc(s_pre, 16)
                for ci, ch in enumerate(p1_chunks):
                    tau, lo, w = ch["tau"], ch["lo"], ch["w"]
                    ti = tag_order.index(tau)
                    if ch["new_tag"]:
                        if ti >= 2:
                            # wih slot reuse: PE done with all groups of the
                            # tag two back
                            sync.wait_ge(s1_mm, groups_before_tag[tag_order[ti - 1]])
                        sync.dma_start(
                            wih_sb[:, ch["wih_slot"] * WIH_W:(ch["wih_slot"] + 1) * WIH_W],
                            wih_d[tau, :, :],
                        ).then_inc(wih_sem[ti % 2], 16)
                    if ci >= 2:
                        # emb slot reuse: PE done with groups of chunk ci-2
                        sync.wait_ge(s1_mm, NMT * (ci - 1))
                    eslot = ci % 2
                    p_lo = ch["pos_base"] + lo // BL
                    npos = w // BL
                    for kc in range(3):
                        n = KT[kc]
                        sync.dma_start(
                            emb_sb.ap()
                            .rearrange("p (s c b) -> p s c b", s=2, c=3)[
                                :n, eslot, kc, 0:w]
                            .rearrange("p (s b) -> p s b", b=BL),
                            embt_d[p_lo:p_lo + npos, 128 * kc:128 * kc + n, :]
                            .rearrange("s p b -> p s b"),
                        ).then_inc(emb_sem[ci % 2], 16)
                    if ci >= 2:
                        co = p1_chunks[ci - 2]
                        sync.wait_ge(s1_cp, NMT * (ci - 1))
                        p_lo2 = co["pos_base"] + co["lo"] // BL
                        npos2 = co["w"] // BL
                        sync.dma_start(
                            zx_d[p_lo2:p_lo2 + npos2, :, :]
                            .rearrange("s p mb -> p s mb"),
                            zxstage.ap()
                            .rearrange("p (e s mb) -> p e s mb", e=2, s=NMT)[
                                :, (ci - 2) % 2, 0:npos2, :],
                        ).then_inc(out_sem[(ci - 2) % 2], 16)
                # drain tail
                for ci in range(max(0, NCH - 2), NCH):
                    co = p1_chunks[ci]
                    sync.wait_ge(s1_cp, NMT * (ci + 1))
                    p_lo2 = co["pos_base"] + co["lo"] // BL
                    npos2 = co["w"] // BL
                    sync.dma_start(
                        zx_d[p_lo2:p_lo2 + npos2, :, :]
                        .rearrange("s p mb -> p s mb"),
                        zxstage.ap()
                        .rearrange("p (e s mb) -> p e s mb", e=2, s=NMT)[
                            :, ci % 2, 0:npos2, :],
                    ).then_inc(out_sem[ci % 2], 16)

            @blockA.tensor
            def _(tensor):
                gidx = 0
                for ci, ch in enumerate(p1_chunks):
                    tau, lo, w = ch["tau"], ch["lo"], ch["w"]
                    ti = tag_order.index(tau)
                    tensor.wait_ge(wih_sem[ti % 2], 16 * (ti // 2 + 1))
                    tensor.wait_ge(emb_sem[ci % 2], 48 * (ci // 2 + 1))
                    slot, eslot = ch["wih_slot"], ci % 2
                    for m in range(NMT):
                        bank = ps1[gidx % 4]
                        if gidx >= 4:
                            tensor.wait_ge(s1_cp, gidx - 3)
                        for kc in range(3):
                            n = KT[kc]
                            mm = tensor.matmul(
                                bank[:, 0:w],
                                wih_sb[:n, slot * WIH_W + (kc * 16 + m) * 128:
                                       slot * WIH_W + (kc * 16 + m) * 128 + 128],
                                emb_sb[:n, eslot * 3 * CHUNK + kc * CHUNK:
                                       eslot * 3 * CHUNK + kc * CHUNK + w],
                                start=(kc == 0),
                                stop=(kc == 2),
                            )
                        mm.then_inc(s1_mm, 1)
                        gidx += 1

            @blockA.vector
            def _(vector):
                gidx = 0
                for ci, ch in enumerate(p1_chunks):
                    eslot = ci % 2
                    for m in range(NMT):
                        vector.wait_ge(s1_mm, gidx + 1)
                        if ci >= 2 and m == 0:
                            vector.wait_ge(out_sem[ci % 2], 16 * ((ci - 2) // 2 + 1))
                        npos = ch["w"] // BL
                        vector.tensor_copy(
                            zxstage.ap()
                            .rearrange("p (e s mb) -> p e s mb", e=2, s=NMT)[
                                :, eslot, 0:npos, m * BL:m * BL + BL],
                            ps1[gidx % 4]
                            .ap()
                            .rearrange("p (s b) -> p s b", b=BL)[:, 0:npos, :],
                        ).then_inc(s1_cp, 1)
                        gidx += 1

    # =================== Block B: phase 2 ===================
    with contextlib.ExitStack() as p2ctx:
        pg = [[p2ctx.enter_context(nc.psum_tensor(f"pg{e}_{q}", [128, 128], F32))
               for q in range(4)] for e in range(2)]
        whh_str = p2ctx.enter_context(
            nc.sbuf_tensor("whh_str", [128, N_STREAM * BANK_W], BF16))
        zx_sb = p2ctx.enter_context(nc.sbuf_tensor("zx_sb", [128, 2 * 512], BF16))

        def whh_src(t):
            tau = tags[t]
            if tau in resident:
                return whh_res, resident[tau] * BANK_W
            return whh_str, (miss_idx_of_step[t] % N_STREAM) * BANK_W

        with nc.Block() as blockB:
            @blockB.sync
            def _(sync):
                for sl in range(2):
                    if NCH - 2 + sl >= 0 and NCH > sl:
                        last_ci = max(c2 for c2 in range(NCH) if c2 % 2 == sl)
                        sync.wait_ge(out_sem[sl], 16 * (last_ci // 2 + 1))
                for kind, v in ld_seq:
                    if kind == "zx":
                        t = v
                        if t >= 2:
                            sync.wait_ge(s_zxd, t - 1)
                        sync.dma_start(
                            zx_sb[:, (t % 2) * 512:(t % 2) * 512 + 512],
                            zx_d[pos_of_step[t], :, :],
                        ).then_inc(zx_sem[t % 2], 16)
                    else:
                        j = v
                        ms = miss_steps[j]
                        if j >= N_STREAM:
                            sync.wait_ge(s_gates, miss_steps[j - N_STREAM] + 1)
                        sync.dma_start(
                            whh_str[:, (j % N_STREAM) * BANK_W:
                                    (j % N_STREAM + 1) * BANK_W],
                            whh_d[tags[ms], :, :],
                        ).then_inc(miss_sem[j % N_STREAM], 16)
                sync.wait_ge(s_hnew, S)
                sync.wait_ge(s_c, S)
                sync.dma_start(hout_d[:, :], hT[:, :]).then_inc(s_fin, 16)
                sync.dma_start(cout_d[:, :], c_st[:, :]).then_inc(s_fin, 16)
                sync.wait_ge(s_fin, 32)

            @blockB.tensor
            def _(tensor):
                for t in range(S):
                    src, cbase = whh_src(t)
                    tensor.wait_ge(zx_sem[t % 2], 16 * (t // 2 + 1))
                    if t in miss_idx_of_step:
                        j = miss_idx_of_step[t]
                        tensor.wait_ge(miss_sem[j % N_STREAM],
                                       16 * (j // N_STREAM + 1))
                    if t == 0:
                        tensor.wait_ge(s_pre, n_pre)
                    if t >= 2:
                        tensor.wait_ge(s_pfree, t - 1)
                    banks = pg[t % 2]
                    for q in range(4):
                        mmz = tensor.matmul(
                            banks[q][:, :],
                            ident_sb[:, :],
                            zx_sb[:, (t % 2) * 512 + 128 * q:
                                  (t % 2) * 512 + 128 * q + 128],
                            start=True, stop=False, skip_group_check=True,
                        )
                        if q == 3:
                            mmz.then_inc(s_zxd, 1)
                    if t >= 1:
                        tensor.wait_ge(s_hnew, t)
                    for m in range(NMT):
                        for kc in range(NKT):
                            last = (m == NMT - 1 and kc == NKT - 1)
                            mm = tensor.matmul(
                                banks[m // 4][:, 32 * (m % 4):32 * (m % 4) + 32],
                                src[:, cbase + (kc * 16 + m) * 128:
                                    cbase + (kc * 16 + m) * 128 + 128],
                                hT[:, 32 * kc:32 * kc + 32],
                                start=False, stop=last, skip_group_check=True,
                            )
                            if kc == NKT - 1:
                                if m == 3:
                                    mm.then_inc(s_g, 1)
                                elif m == 7:
                                    mm.then_inc(s_i2, 1)
                                elif m == 11:
                                    mm.then_inc(s_if, 1)
                                elif m == NMT - 1:
                                    mm.then_inc(s_gates, 1)

            @blockB.scalar
            def _(scalar):
                for t in range(S):
                    banks = pg[t % 2]
                    scalar.wait_ge(s_g, t + 1)
                    scalar.activation(tg[:, :], banks[0][:, :], TANH).then_inc(s_act, 1)
                    scalar.wait_ge(s_i2, t + 1)
                    scalar.activation(sig_if[:, 0:128], banks[1][:, :], SIG).then_inc(s_act, 1)
                    scalar.wait_ge(s_if, t + 1)
                    scalar.activation(sig_if[:, 128:256], banks[2][:, :], SIG).then_inc(s_act, 1)
                    scalar.wait_ge(s_c, t + 1)
                    scalar.activation(tanhc[:, :], c_st[:, :], TANH).then_inc(s_tc, 1)
                    scalar.wait_ge(s_gates, t + 1)
                    scalar.activation(sig_o[:, :], banks[3][:, :], SIG).then_inc(s_pfree, 1)

            @blockB.vector
            def _(vector):
                for t in range(S):
                    vector.wait_ge(s_act, 3 * t + 2)
                    vector.tensor_mul(tmp2[:, :], sig_if[:, 0:128], tg[:, :])
                    vector.wait_ge(s_act, 3 * t + 3)
                    vector.tensor_mul(tmp1[:, :], sig_if[:, 128:256], c_st[:, :])
                    vector.tensor_add(c_st[:, :], tmp1[:, :], tmp2[:, :]).then_inc(s_c, 1)
                    vector.wait_ge(s_tc, t + 1)
                    vector.wait_ge(s_pfree, t + 1)
                    vector.tensor_mul(hT[:, :], sig_o[:, :], tanhc[:, :]).then_inc(s_hnew, 1)

    ctx.close()
    return nc, {"pos_of_step": pos_of_step, "S": S}


_CACHE = {}


def _get_program(tags_key):
    if tags_key not in _CACHE:
        _CACHE[tags_key] = build_program(list(tags_key))
    return _CACHE[tags_key]


def run_on_hw(x, tags, emb, W_ih, W_hh, b_ih, b_hh, h0, c0, trace=False):
    tags_key = tuple(int(t) for t in tags)
    nc, meta = _get_program(tags_key)
    pos = meta["pos_of_step"]
    in_maps = [
        host_prep(c, x, tags, emb, W_ih, W_hh, b_ih, b_hh, h0, c0, pos)
        for c in range(NCORES)
    ]
    res = bass_utils.run_bass_kernel_spmd(
        nc, in_maps, core_ids=list(range(NCORES)), trace=trace
    )
    S = meta["S"]
    h = np.zeros((B, H), dtype=np.float32)
    c = np.zeros((B, H), dtype=np.float32)
    for core in range(NCORES):
        hp = np.asarray(res.results[core]["hout"]).astype(np.float32)
        cp = np.asarray(res.results[core]["cout"])
        bs = slice(BL * core, BL * (core + 1))
        for k in range(4):
            h[bs, 128 * k:128 * (k + 1)] = hp[:, 32 * k:32 * k + BL].T
            c[bs, 128 * k:128 * (k + 1)] = cp[:, 32 * k:32 * k + BL].T
    return h, c, res


def kernel(x, tags, emb, W_ih, W_hh, b_ih, b_hh, fc_W, fc_b, h0, c0):
    x = np.asarray(x)
    tags = np.asarray(tags)
    emb = np.asarray(emb, dtype=np.float32)
    W_ih = np.asarray(W_ih, dtype=np.float32)
    W_hh = np.asarray(W_hh, dtype=np.float32)
    b_ih = np.asarray(b_ih, dtype=np.float32)
    b_hh = np.asarray(b_hh, dtype=np.float32)
    fc_W = np.asarray(fc_W, dtype=np.float32)
    fc_b = np.asarray(fc_b, dtype=np.float32)
    h0 = np.asarray(h0, dtype=np.float32)
    c0 = np.asarray(c0, dtype=np.float32)
    h, c, _ = run_on_hw(x, tags, emb, W_ih, W_hh, b_ih, b_hh, h0, c0)
    out = (1.0 / (1.0 + np.exp(-(h @ fc_W.T + fc_b)))).astype(np.float32)
    return out, h, c
